# revision 7
# baseline (speedup 1.0000x reference)
# DeepGEMM-style fp8 block-quantized linear for Trainium2, 8-core SPMD.
#
# reference semantics:
#   x_dq = dequant(quant_e4m3fn(x, per-token per-128-group amax/448 scales))
#   w_dq = w_fp8 * w_scale (per 128x128 block)
#   out  = (x_dq @ w_dq.T).astype(bf16)          # fp32 accumulation
#
# Strategy (per core; 2x4 [M x N] grid => M2=2048, N2=1024 per core):
#   - TRN fp8_e4m3 tops out at 240 (vs OCP e4m3fn's 448), so quantize
#     x * (112/amax) on TRN's grid: identical rounding for normals (pure
#     exponent shift); dequantize with s4 = amax/112.
#   - scales folded into fp16 matmul operands (fp8 matmul is out of the
#     2e-2 error budget; fp16 error is dominated by PSUM fp22 accum).
#   - warmup is column-major: W load-pair c and X chunk c of the first
#     `warm` m-tiles cover the same 14 k-blocks, so each column's W
#     dequant+transpose, X quant pipeline, and band matmuls (kq 2c,2c+1
#     x warm m-tiles into 2*warm psum banks) pace the W HBM stream.
#   - the gpsimd queue carries only DMA dispatches (W cast-loads, xn
#     loads) plus carefully-placed dequants so load descriptors are
#     never stuck behind compute (FIFO head-of-line).
#   - ws-dequant applied post-transpose on wt tiles: ACT per-kb slices
#     (nb0-3, per-partition scale) and DVE bcast multiplies (nb4-7).
#   - X: amax/scales/quant on DVE; dequant on V/ACT/G (band) or G
#     (steady); transposes on sync; phase-batched to avoid convoys.

import numpy as np
import ml_dtypes
from contextlib import ExitStack

import concourse.bass as bass
import concourse.mybir as mybir
import concourse.tile as tile
from concourse import bacc
from concourse.bass_utils import run_bass_kernel_spmd

dt = mybir.dt

M, N, K = 4096, 4096, 7168
MSH, NSH = 2, 4                     # core grid: 2 along M, 4 along N
NCORES = MSH * NSH
BLK = 128


def bcast_inner(ap, n):
    """Append a step-0 inner dim of size n (free-dim broadcast read)."""
    return bass.AP(tensor=ap.tensor, offset=ap.offset, ap=[*ap.ap, [0, n]])


def emit_kernel(ctx, tc, o_d, x_d, w_d, ws_d, *, nqw=512, warm=3):
    nc = tc.nc
    f32, f16, f8 = dt.float32, dt.float16, dt.float8e4
    bf16 = dt.bfloat16
    M2, Kd = x_d.shape
    N2, _ = w_d.shape
    KB = Kd // BLK              # 56 k-blocks
    NB = N2 // BLK              # 8 n-blocks
    MT = M2 // BLK              # 16 m-tiles
    NQ = N2 // nqw              # psum tiles per m-tile
    KQW = 8                     # W k-groups (one wt tile each)
    KBW = KB // KQW             # 7 kb per W group
    WP = 4                      # W load pairs == X chunks per m-tile
    KWL = Kd // WP              # 1792
    KQ = 4                      # x chunks per m-tile
    KL = Kd // KQ               # 1792
    KBQ = KB // KQ              # 14

    wtp = ctx.enter_context(tc.tile_pool(name="wt", bufs=KQW))
    constp = ctx.enter_context(tc.tile_pool(name="consts", bufs=1))
    wqp = ctx.enter_context(tc.tile_pool(name="wq", bufs=3))
    xnp = ctx.enter_context(tc.tile_pool(name="xn", bufs=5))
    scp = ctx.enter_context(tc.tile_pool(name="sc", bufs=8))
    xqp = ctx.enter_context(tc.tile_pool(name="xq", bufs=2))
    xdqp = ctx.enter_context(tc.tile_pool(name="xdq", bufs=2))
    xtp = ctx.enter_context(tc.tile_pool(name="xt", bufs=KQ * warm + 1))
    obp = ctx.enter_context(tc.tile_pool(name="ob", bufs=2))
    psp = ctx.enter_context(tc.tile_pool(name="ps", bufs=8, space="PSUM"))

    # w_scale broadcast across partitions via step-0 partition DMA read
    wsb = constp.tile([128, NB * KB], f32)
    ws_flat = ws_d.rearrange("a b -> (a b)")
    ws_b = bass.AP(tensor=ws_flat.tensor, offset=ws_flat.offset,
                   ap=[[0, 128], *ws_flat.ap])
    nc.gpsimd.dma_start(wsb[:], ws_b)

    # --- W pipeline ---
    wts = [None] * KQW
    wq_pend = {}

    def emit_w_loads(p):
        wqs = []
        for nb in range(NB):
            wq = wqp.tile([128, KWL], f16, tag="wq", name=f"wq{p}_{nb}")
            nc.gpsimd.dma_start(
                wq[:], w_d[nb * BLK:(nb + 1) * BLK, p * KWL:(p + 1) * KWL])
            wqs.append(wq)
        wq_pend[p] = wqs

    def emit_w_compute(p):
        wqs = wq_pend.pop(p)
        for h in range(2):
            kq = 2 * p + h
            wts[kq] = wtp.tile([128, KBW, N2], f16, tag="wt", name=f"wt{kq}")
        for nb in range(NB):
            for h in range(2):
                kq = 2 * p + h
                nc.sync.dma_start(
                    wts[kq][:, :, nb * BLK:(nb + 1) * BLK],
                    wqs[nb][:, h * (KWL // 2):(h + 1) * (KWL // 2)],
                    transpose=True)
        for h in range(2):
            kq = 2 * p + h
            wt = wts[kq]
            for nb in range(NB):
                if nb < 4:      # ACT: per-kb [128,128] with partition scale
                    for j in range(KBW):
                        kb = kq * KBW + j
                        sl = wt[:, j, nb * BLK:(nb + 1) * BLK]
                        nc.scalar.mul(sl, sl,
                                      wsb[:, nb * KB + kb:nb * KB + kb + 1])
                else:           # DVE: one fused bcast multiply per nb
                    sl = wt[:, :, nb * BLK:(nb + 1) * BLK]
                    nc.vector.tensor_tensor(
                        out=sl, in0=sl,
                        in1=bcast_inner(
                            wsb[:, nb * KB + kq * KBW:
                                   nb * KB + (kq + 1) * KBW], BLK),
                        op=mybir.AluOpType.mult)

    # --- X pipeline ---
    xts = {mt: {} for mt in range(MT)}
    xn_pend = {}

    def emit_x_loads(items):
        for (mt, c) in items:
            xn = xnp.tile([128, KL], bf16, tag="xn", name=f"xn{mt}_{c}")
            nc.gpsimd.dma_start(
                xn[:], x_d[mt * BLK:(mt + 1) * BLK, c * KL:(c + 1) * KL])
            xn_pend[(mt, c)] = xn

    def emit_x_compute(items, dq_engs):
        """Phase-batched quant pipeline for a list of (mt, chunk).
        dq_engs: per-item dequant engine: 'v', 'a' (per-kb), or 'g'."""
        xns = [xn_pend.pop(it) for it in items]
        amaxs, s4s, inv4s, xqs = [], [], [], []
        for i, (mt, c) in enumerate(items):
            xng = xns[i][:].rearrange("p (kb c) -> p kb c", c=BLK)
            amax = scp.tile([128, KBQ], f32, tag="amax", name=f"am{mt}_{c}")
            nc.vector.reduce_max(
                amax[:], xng, axis=mybir.AxisListType.X,
                apply_absolute_value=True)
            amaxs.append(amax)
        for i, (mt, c) in enumerate(items):
            s4 = scp.tile([128, KBQ], f32, tag="s4", name=f"s4_{mt}_{c}")
            nc.vector.tensor_scalar(
                out=s4[:], in0=amaxs[i][:],
                scalar1=1e-12, scalar2=float(np.float32(1.0 / 112.0)),
                op0=mybir.AluOpType.max, op1=mybir.AluOpType.mult)
            s4s.append(s4)
        for i, (mt, c) in enumerate(items):
            inv4 = scp.tile([128, KBQ], f32, tag="inv4", name=f"iv{mt}_{c}")
            nc.vector.reciprocal(inv4[:], s4s[i][:])
            inv4s.append(inv4)
        for i, (mt, c) in enumerate(items):
            xng = xns[i][:].rearrange("p (kb c) -> p kb c", c=BLK)
            xq = xqp.tile([128, KL], f8, tag="xq", name=f"xq{mt}_{c}")
            xqg = xq[:].rearrange("p (kb c) -> p kb c", c=BLK)
            nc.vector.tensor_tensor(
                out=xqg, in0=xng, in1=bcast_inner(inv4s[i][:], BLK),
                op=mybir.AluOpType.mult)
            xqs.append(xq)
        for i, (mt, c) in enumerate(items):
            xqg = xqs[i][:].rearrange("p (kb c) -> p kb c", c=BLK)
            xdq = xdqp.tile([128, KL], f16, tag="xdq", name=f"xd{mt}_{c}")
            xdqg = xdq[:].rearrange("p (kb c) -> p kb c", c=BLK)
            e = dq_engs[i]
            if e == 'a':
                for j in range(KBQ):
                    nc.scalar.mul(xdqg[:, j, :], xqg[:, j, :],
                                  s4s[i][:, j:j + 1])
            else:
                eng = nc.vector if e == 'v' else nc.gpsimd
                eng.tensor_tensor(
                    out=xdqg, in0=xqg, in1=bcast_inner(s4s[i][:], BLK),
                    op=mybir.AluOpType.mult)
            xt_t = xtp.tile([128, KBQ, 128], f16, tag="xt",
                            name=f"xt{mt}_{c}")
            nc.sync.dma_start(xt_t[:], xdq[:], transpose=True)
            xts[mt][c] = xt_t

    def emit_x(mt):
        items = [(mt, c) for c in range(KQ)]
        emit_x_loads(items)
        emit_x_compute(items, ['g'] * KQ)

    def mm(ps, mt, kb, q, start, stop):
        nc.tensor.matmul(
            ps[:],
            xts[mt][kb // KBQ][:, kb % KBQ, :],
            wts[kb // KBW][:, kb % KBW, q * nqw:(q + 1) * nqw],
            start=start, stop=stop)

    def emit_evac(mt, ps_tiles):
        ob = obp.tile([128, N2], bf16, tag="ob", name=f"ob{mt}")
        for q in range(NQ):
            nc.scalar.copy(ob[:, q * nqw:(q + 1) * nqw], ps_tiles[q][:])
        nc.scalar.dma_start(o_d[mt * BLK:(mt + 1) * BLK, :], ob[:])

    # ---- emission schedule ----
    emit_w_loads(0)
    emit_x_loads([(mt, 0) for mt in range(warm)])
    emit_w_loads(1)

    psw = [[psp.tile([128, nqw], f32, tag="ps", name=f"psw{mt}_{q}")
            for q in range(NQ)] for mt in range(warm)]
    # column-major warmup: W pair c + X chunk c (warm m-tiles) + band MMs.
    # dequants: mt0 -> DVE, mt1 -> ACT, mt2 -> GpSimd (the 'g' dequant
    # doubles as the FIFO spacer that delays the next W load dispatch to
    # exactly when its staging buffers free up).
    for c in range(WP):
        emit_w_compute(c)
        emit_x_compute([(mt, c) for mt in range(warm)],
                       ['v', 'a', 'g'][:warm])
        if c + 2 < WP:
            emit_w_loads(c + 2)
        if c + 1 < WP:
            emit_x_loads([(mt, c + 1) for mt in range(warm)])
        for kq in (2 * c, 2 * c + 1):
            for mt in range(warm):
                for j in range(KBW):
                    kb = kq * KBW + j
                    for q in range(NQ):
                        mm(psw[mt][q], mt, kb, q,
                           start=(kb == 0), stop=(kb == KB - 1))
        if c == 2 and warm < MT:
            emit_x(warm)
        if c == 3 and warm + 1 < MT:
            emit_x(warm + 1)
    for mt in range(warm):
        emit_evac(mt, psw[mt])

    # steady state
    for mt in range(warm, MT):
        la = mt + 2
        if la < MT and not xts[la]:
            emit_x(la)
        pst = [psp.tile([128, nqw], f32, tag="ps", name=f"ps{mt}_{q}")
               for q in range(NQ)]
        for kb in range(KB):
            for q in range(NQ):
                mm(pst[q], mt, kb, q, start=(kb == 0), stop=(kb == KB - 1))
        emit_evac(mt, pst)


def build_nc(m2, n2, k, **kw):
    nc = bacc.Bacc("TRN2", target_bir_lowering=False, debug=False, num_devices=NCORES)
    x_d = nc.dram_tensor("x", [m2, k], dt.bfloat16, kind="ExternalInput").ap()
    w_d = nc.dram_tensor("w", [n2, k], dt.float32, kind="ExternalInput").ap()
    ws_d = nc.dram_tensor("ws", [n2 // BLK, k // BLK], dt.float32, kind="ExternalInput").ap()
    o_d = nc.dram_tensor("o", [m2, n2], dt.bfloat16, kind="ExternalOutput").ap()
    with tile.TileContext(nc) as tc, ExitStack() as ctx:
        emit_kernel(ctx, tc, o_d, x_d, w_d, ws_d, **kw)
    nc.compile()
    return nc


_cache = {}


def _get_nc():
    if "nc" not in _cache:
        _cache["nc"] = build_nc(M // MSH, N // NSH, K)
    return _cache["nc"]


def kernel(input, weight_fp8, weight_scale, _trace=False, _trace_kwargs=None):
    input = np.asarray(input)
    if input.dtype != ml_dtypes.bfloat16:
        input = input.astype(ml_dtypes.bfloat16)
    weight_fp8 = np.asarray(weight_fp8, dtype=np.float32)
    weight_scale = np.asarray(weight_scale, dtype=np.float32)
    M2, N2 = M // MSH, N // NSH
    NSB = N2 // BLK

    in_maps = []
    for c in range(NCORES):
        mi, ni = divmod(c, NSH)
        in_maps.append({
            "x": np.ascontiguousarray(input[mi * M2:(mi + 1) * M2]),
            "w": np.ascontiguousarray(weight_fp8[ni * N2:(ni + 1) * N2]),
            "ws": np.ascontiguousarray(weight_scale[ni * NSB:(ni + 1) * NSB]),
        })

    nc = _get_nc()
    kw = {}
    if _trace:
        kw = dict(trace=True, **(_trace_kwargs or {}))
    res = run_bass_kernel_spmd(nc, in_maps, core_ids=list(range(NCORES)), **kw)

    out = np.empty((M, N), dtype=ml_dtypes.bfloat16)
    for c in range(NCORES):
        mi, ni = divmod(c, NSH)
        out[mi * M2:(mi + 1) * M2, ni * N2:(ni + 1) * N2] = res.results[c]["o"]
    if _trace:
        return out, res
    return out


# revision 10
# speedup vs baseline: 1.0260x; 1.0260x over previous
# DeepGEMM-style fp8 block-quantized linear for Trainium2, 8-core SPMD.
#
# reference semantics:
#   x_dq = dequant(quant_e4m3fn(x, per-token per-128-group amax/448 scales))
#   w_dq = w_fp8 * w_scale (per 128x128 block)
#   out  = (x_dq @ w_dq.T).astype(bf16)          # fp32 accumulation
#
# Strategy (per core; 2x4 [M x N] grid => M2=2048, N2=1024 per core):
#   - TRN fp8_e4m3 tops out at 240 (vs OCP e4m3fn's 448), so quantize
#     x * (112/amax) on TRN's grid: identical rounding for normals (pure
#     exponent shift); dequantize with s4 = amax/112.
#   - scales folded into fp16 matmul operands (fp8 matmul is out of the
#     2e-2 error budget; fp16 error is dominated by PSUM fp22 accum).
#   - W loads are RAW f32 (cast-during-DMA runs at ~1/4 HBM speed); the
#     e4m3fn payload lives exactly in the high 2 bytes of each f32, so
#     a strided bf16 view + one fused multiply does extract + ws-dequant
#     + f16 cast in a single engine pass, split ACT/DVE/GpSimd.
#   - W streamed k-major per kq group (8 wt tiles) so matmuls gate at
#     kq granularity; warmup band sweeps kq-major across 2*warm psum
#     banks, rate-matched to the full-speed W HBM stream (~82us).
#   - X pipeline phase-batched (no FIFO convoys): xn loads on gpsimd,
#     amax/recip/quant on DVE, group-scale on GpSimd, dequant split
#     DVE/ACT/GpSimd; all transposes on sync; W loads + psum evac on
#     the scalar queue.

import numpy as np
import ml_dtypes
from contextlib import ExitStack

import concourse.bass as bass
import concourse.mybir as mybir
import concourse.tile as tile
from concourse import bacc
from concourse.bass_utils import run_bass_kernel_spmd

dt = mybir.dt

M, N, K = 4096, 4096, 7168
MSH, NSH = 2, 4                     # core grid: 2 along M, 4 along N
NCORES = MSH * NSH
BLK = 128


def bcast_inner(ap, n):
    """Append a step-0 inner dim of size n (free-dim broadcast read)."""
    return bass.AP(tensor=ap.tensor, offset=ap.offset, ap=[*ap.ap, [0, n]])


def emit_kernel(ctx, tc, o_d, x_d, w_d, ws_d, *, nqw=512, warm=3):
    nc = tc.nc
    f32, f16, f8 = dt.float32, dt.float16, dt.float8e4
    bf16 = dt.bfloat16
    M2, Kd = x_d.shape
    N2, _ = w_d.shape
    KB = Kd // BLK              # 56 k-blocks
    NB = N2 // BLK              # 8 n-blocks
    MT = M2 // BLK              # 16 m-tiles
    NQ = N2 // nqw              # psum tiles per m-tile
    KQW = 8                     # W k-groups (one wt tile each)
    KBW = KB // KQW             # 7 kb per W group
    KWL = Kd // KQW             # 896 k per W chunk
    KQ = 4                      # x chunks per m-tile
    KL = Kd // KQ               # 1792
    KBQ = KB // KQ              # 14

    wtp = ctx.enter_context(tc.tile_pool(name="wt", bufs=KQW))
    constp = ctx.enter_context(tc.tile_pool(name="consts", bufs=1))
    w32p = ctx.enter_context(tc.tile_pool(name="w32", bufs=3))
    w16p = ctx.enter_context(tc.tile_pool(name="w16", bufs=3))
    xnp = ctx.enter_context(tc.tile_pool(name="xn", bufs=4))
    scp = ctx.enter_context(tc.tile_pool(name="sc", bufs=8))
    xqp = ctx.enter_context(tc.tile_pool(name="xq", bufs=2))
    xdqp = ctx.enter_context(tc.tile_pool(name="xdq", bufs=2))
    xtp = ctx.enter_context(tc.tile_pool(name="xt", bufs=KQ * warm + 1))
    obp = ctx.enter_context(tc.tile_pool(name="ob", bufs=2))
    psp = ctx.enter_context(tc.tile_pool(name="ps", bufs=8, space="PSUM"))

    # w_scale broadcast across partitions via step-0 partition DMA read
    wsb = constp.tile([128, NB * KB], f32)
    ws_flat = ws_d.rearrange("a b -> (a b)")
    ws_b = bass.AP(tensor=ws_flat.tensor, offset=ws_flat.offset,
                   ap=[[0, 128], *ws_flat.ap])
    nc.gpsimd.dma_start(wsb[:], ws_b)

    # --- W pipeline ---
    wts = [None] * KQW
    w32_pend = {}

    def emit_w_loads(kq):
        """Raw f32 chunk loads (full HBM speed) on the scalar queue."""
        chunks = []
        for nb in range(NB):
            w32 = w32p.tile([128, KWL], f32, tag="w32", name=f"w32_{kq}_{nb}")
            nc.scalar.dma_start(
                w32[:], w_d[nb * BLK:(nb + 1) * BLK,
                            kq * KWL:(kq + 1) * KWL])
            chunks.append(w32)
        w32_pend[kq] = chunks

    def emit_w_compute(kq):
        wt = wtp.tile([128, KBW, N2], f16, tag="wt", name=f"wt{kq}")
        wts[kq] = wt
        w16s = []
        for nb in range(NB):
            w32 = w32_pend[kq][nb]
            # strided bf16 view of the f32 carrier: the odd u16 halves
            # hold the e4m3fn values exactly (bf16 truncation is exact)
            hi = w32[:].bitcast(bf16).rearrange(
                "p (kb c two) -> p kb c two", two=2, c=BLK)
            w16 = w16p.tile([128, KWL], f16, tag="w16", name=f"w16_{kq}_{nb}")
            w16g = w16[:].rearrange("p (kb c) -> p kb c", c=BLK)
            # no ACT here: the scalar queue is occupied by the W load
            # trickle for the whole band (FIFO HOL / WAR deadlock risk)
            eng = nc.vector if nb < 2 else nc.gpsimd
            eng.tensor_tensor(
                out=w16g, in0=hi[:, :, :, 1],
                in1=bcast_inner(
                    wsb[:, nb * KB + kq * KBW:
                           nb * KB + (kq + 1) * KBW], BLK),
                op=mybir.AluOpType.mult)
            w16s.append(w16)
        del w32_pend[kq]
        for nb in range(NB):
            nc.sync.dma_start(
                wt[:, :, nb * BLK:(nb + 1) * BLK], w16s[nb][:],
                transpose=True)

    # --- X pipeline ---
    xts = {mt: {} for mt in range(MT)}
    xn_pend = {}

    def emit_x_loads(items):
        for (mt, c) in items:
            xn = xnp.tile([128, KL], bf16, tag="xn", name=f"xn{mt}_{c}")
            nc.gpsimd.dma_start(
                xn[:], x_d[mt * BLK:(mt + 1) * BLK, c * KL:(c + 1) * KL])
            xn_pend[(mt, c)] = xn

    def emit_x_compute(items, dq_engs):
        """Phase-batched quant pipeline for a list of (mt, chunk).
        dq_engs: per-item dequant engine: 'v', 'a' (per-kb), or 'g'."""
        xns = [xn_pend.pop(it) for it in items]
        amaxs, s4s, inv4s = [], [], []
        for i, (mt, c) in enumerate(items):
            xng = xns[i][:].rearrange("p (kb c) -> p kb c", c=BLK)
            amax = scp.tile([128, KBQ], f32, tag="amax", name=f"am{mt}_{c}")
            nc.vector.reduce_max(
                amax[:], xng, axis=mybir.AxisListType.X,
                apply_absolute_value=True)
            amaxs.append(amax)
        for i, (mt, c) in enumerate(items):
            s4 = scp.tile([128, KBQ], f32, tag="s4", name=f"s4_{mt}_{c}")
            nc.gpsimd.tensor_scalar(
                out=s4[:], in0=amaxs[i][:],
                scalar1=1e-12, scalar2=float(np.float32(1.0 / 112.0)),
                op0=mybir.AluOpType.max, op1=mybir.AluOpType.mult)
            s4s.append(s4)
        for i, (mt, c) in enumerate(items):
            inv4 = scp.tile([128, KBQ], f32, tag="inv4", name=f"iv{mt}_{c}")
            nc.vector.reciprocal(inv4[:], s4s[i][:])
            inv4s.append(inv4)
        for i, (mt, c) in enumerate(items):
            xng = xns[i][:].rearrange("p (kb c) -> p kb c", c=BLK)
            xq = xqp.tile([128, KL], f8, tag="xq", name=f"xq{mt}_{c}")
            xqg = xq[:].rearrange("p (kb c) -> p kb c", c=BLK)
            nc.vector.tensor_tensor(
                out=xqg, in0=xng, in1=bcast_inner(inv4s[i][:], BLK),
                op=mybir.AluOpType.mult)
            xdq = xdqp.tile([128, KL], f16, tag="xdq", name=f"xd{mt}_{c}")
            xdqg = xdq[:].rearrange("p (kb c) -> p kb c", c=BLK)
            e = dq_engs[i]
            if e == 'a':
                for j in range(KBQ):
                    nc.scalar.mul(xdqg[:, j, :], xqg[:, j, :],
                                  s4s[i][:, j:j + 1])
            else:
                eng = nc.vector if e == 'v' else nc.gpsimd
                eng.tensor_tensor(
                    out=xdqg, in0=xqg, in1=bcast_inner(s4s[i][:], BLK),
                    op=mybir.AluOpType.mult)
            xt_t = xtp.tile([128, KBQ, 128], f16, tag="xt",
                            name=f"xt{mt}_{c}")
            nc.sync.dma_start(xt_t[:], xdq[:], transpose=True)
            xts[mt][c] = xt_t

    def emit_x(mt):
        items = [(mt, c) for c in range(KQ)]
        emit_x_loads(items)
        emit_x_compute(items, ['a', 'a', 'g', 'g'])

    def mm(ps, mt, kb, q, start, stop):
        nc.tensor.matmul(
            ps[:],
            xts[mt][kb // KBQ][:, kb % KBQ, :],
            wts[kb // KBW][:, kb % KBW, q * nqw:(q + 1) * nqw],
            start=start, stop=stop)

    def emit_evac(mt, ps_tiles):
        ob = obp.tile([128, N2], bf16, tag="ob", name=f"ob{mt}")
        for q in range(NQ):
            nc.scalar.copy(ob[:, q * nqw:(q + 1) * nqw], ps_tiles[q][:])
        nc.scalar.dma_start(o_d[mt * BLK:(mt + 1) * BLK, :], ob[:])

    # ---- emission schedule ----
    for kq in range(KQW):
        emit_w_loads(kq)
    emit_x_loads([(mt, 0) for mt in range(warm)])

    psw = [[psp.tile([128, nqw], f32, tag="ps", name=f"psw{mt}_{q}")
            for q in range(NQ)] for mt in range(warm)]
    # kq-major warmup band: W group kq + (every other kq) the matching
    # X chunk column for the warm m-tiles, then that kq's matmuls.
    for kq in range(KQW):
        emit_w_compute(kq)
        if kq % 2 == 0:
            c = kq // 2
            emit_x_compute([(mt, c) for mt in range(warm)],
                           ['v', 'v', 'g'][:warm])
            if c + 1 < KQ:
                emit_x_loads([(mt, c + 1) for mt in range(warm)])
        for mt in range(warm):
            for j in range(KBW):
                kb = kq * KBW + j
                for q in range(NQ):
                    mm(psw[mt][q], mt, kb, q,
                       start=(kb == 0), stop=(kb == KB - 1))
        if kq == 4 and warm < MT:
            emit_x(warm)
        if kq == 6 and warm + 1 < MT:
            emit_x(warm + 1)
    for mt in range(warm):
        emit_evac(mt, psw[mt])

    # steady state
    for mt in range(warm, MT):
        la = mt + 2
        if la < MT and not xts[la]:
            emit_x(la)
        pst = [psp.tile([128, nqw], f32, tag="ps", name=f"ps{mt}_{q}")
               for q in range(NQ)]
        for kb in range(KB):
            for q in range(NQ):
                mm(pst[q], mt, kb, q, start=(kb == 0), stop=(kb == KB - 1))
        emit_evac(mt, pst)


def build_nc(m2, n2, k, **kw):
    nc = bacc.Bacc("TRN2", target_bir_lowering=False, debug=False, num_devices=NCORES)
    x_d = nc.dram_tensor("x", [m2, k], dt.bfloat16, kind="ExternalInput").ap()
    w_d = nc.dram_tensor("w", [n2, k], dt.float32, kind="ExternalInput").ap()
    ws_d = nc.dram_tensor("ws", [n2 // BLK, k // BLK], dt.float32, kind="ExternalInput").ap()
    o_d = nc.dram_tensor("o", [m2, n2], dt.bfloat16, kind="ExternalOutput").ap()
    with tile.TileContext(nc) as tc, ExitStack() as ctx:
        emit_kernel(ctx, tc, o_d, x_d, w_d, ws_d, **kw)
    nc.compile()
    return nc


_cache = {}


def _get_nc():
    if "nc" not in _cache:
        _cache["nc"] = build_nc(M // MSH, N // NSH, K)
    return _cache["nc"]


def kernel(input, weight_fp8, weight_scale, _trace=False, _trace_kwargs=None):
    input = np.asarray(input)
    if input.dtype != ml_dtypes.bfloat16:
        input = input.astype(ml_dtypes.bfloat16)
    weight_fp8 = np.asarray(weight_fp8, dtype=np.float32)
    weight_scale = np.asarray(weight_scale, dtype=np.float32)
    M2, N2 = M // MSH, N // NSH
    NSB = N2 // BLK

    in_maps = []
    for c in range(NCORES):
        mi, ni = divmod(c, NSH)
        in_maps.append({
            "x": np.ascontiguousarray(input[mi * M2:(mi + 1) * M2]),
            "w": np.ascontiguousarray(weight_fp8[ni * N2:(ni + 1) * N2]),
            "ws": np.ascontiguousarray(weight_scale[ni * NSB:(ni + 1) * NSB]),
        })

    nc = _get_nc()
    kw = {}
    if _trace:
        kw = dict(trace=True, **(_trace_kwargs or {}))
    res = run_bass_kernel_spmd(nc, in_maps, core_ids=list(range(NCORES)), **kw)

    out = np.empty((M, N), dtype=ml_dtypes.bfloat16)
    for c in range(NCORES):
        mi, ni = divmod(c, NSH)
        out[mi * M2:(mi + 1) * M2, ni * N2:(ni + 1) * N2] = res.results[c]["o"]
    if _trace:
        return out, res
    return out
